# revision 19
# baseline (speedup 1.0000x reference)
"""KNN top-16 kernel for Trainium2 (8 NeuronCores, SPMD data-parallel).

Problem: points [4, 8192, 3] fp32 -> nn_idx [4, 8192, 16] int32
(indices of the 16 nearest neighbors by squared L2 distance, jax.lax.top_k
tie semantics: equal values ranked by ascending index).

Numerics — BIT-EXACT vs the reference backend:
The reference adj = sq + (-2 einsum) + sq^T is computed by the neuron
backend as a PE fp32 matmul (ein), then fl(-2 ein), then two left-to-right
fp32 adds.  This kernel reproduces every rounding step exactly:
  PE   : ein panel   = fp32 matmul [x,y,z]_q^T @ [x,y,z]_c   (bitwise == ref)
  ACT  : -t1         = fl(2*ein - sq_i)    Identity, scale=+2, bias=-sq_i,
                       PSUM -> SBUF        (= -fl(sq_i + inner), bitwise)
  POOL : v           = fl(-t1 - sq_j)      gpsimd tensor_tensor subtract,
                       SBUF -> SBUF        (= -fl(t1 + sq_j) = -adj, bitwise)
  DVE  : per-1024-chunk InstMax top-8 + InstMaxIndex, 64-wide merge to
         top-16 values/positions (exact jax tie semantics)
  POOL : (phase B, after one library switch) two local_scatters per block
         route global candidate indices to their rank slots
Sharding: core k handles batch k//2, query rows (k%2)*4096 ... +4096.
No collectives; full inputs in, full output gathered on host.
"""

import numpy as np
from contextlib import ExitStack

B = 4
N = 8192
K = 16
NQ = 4096          # query rows per core
CH = 1024          # candidate chunk width for DVE top-8
NCH = N // CH      # 8 chunks
NBLK = NQ // 128   # 32 blocks of 128 query rows
NEGBIG = -3.0e38

_cache = {}


def _get_nc():
    if "nc" in _cache:
        return _cache["nc"]

    import concourse.bass as bass
    import concourse.bacc as bacc
    import concourse.mybir as mybir
    import concourse.tile as tile

    F32 = mybir.dt.float32
    U16 = mybir.dt.uint16
    I16 = mybir.dt.int16
    I32 = mybir.dt.int32

    nc = bacc.Bacc("TRN2", num_devices=8)

    dLQ = nc.dram_tensor("LQ", [3, NQ], F32, kind="ExternalInput")    # x,y,z queries
    dRC = nc.dram_tensor("RC", [3, N], F32, kind="ExternalInput")     # x,y,z candidates
    dSQI = nc.dram_tensor("SQI", [128, NBLK], F32, kind="ExternalInput")  # -sq_i
    dNSQ = nc.dram_tensor("NSQ", [1, N], F32, kind="ExternalInput")   # +sq_j row
    dCB = nc.dram_tensor("CB", [128, 64], U16, kind="ExternalInput")
    dRK = nc.dram_tensor("RK", [128, 16], I16, kind="ExternalInput")
    dOUT = nc.dram_tensor("OUT", [NQ, K], U16, kind="ExternalOutput")

    with tile.TileContext(nc) as tc, ExitStack() as ctx:
        pool = ctx.enter_context(tc.tile_pool(name="pool", bufs=1))
        rowp = ctx.enter_context(tc.tile_pool(name="rowp", bufs=2))
        psum = ctx.enter_context(tc.tile_pool(name="psum", bufs=2, space="PSUM"))
        small = ctx.enter_context(tc.tile_pool(name="small", bufs=3))

        tLQ = pool.tile([3, NQ], F32)
        nc.sync.dma_start(tLQ[:], dLQ[:])
        tRC = pool.tile([3, N], F32)
        nc.sync.dma_start(tRC[:], dRC[:])
        tSQI = pool.tile([128, NBLK], F32)
        nc.sync.dma_start(tSQI[:], dSQI[:])
        tNSQrow = pool.tile([1, N], F32)
        nc.sync.dma_start(tNSQrow[:], dNSQ[:])
        tONE = pool.tile([1, 128], F32)
        nc.vector.memset(tONE[:], 1.0)
        tCB = pool.tile([128, 64], U16)
        nc.sync.dma_start(tCB[:], dCB[:])
        tRK = pool.tile([128, 16], I16)
        nc.sync.dma_start(tRK[:], dRK[:])

        # replicate +sq_j across all 128 partitions: ones[1,128]^T @ sq[1,512]
        tNSQ = pool.tile([128, N], F32)
        for cc in range(N // 512):
            psn = psum.tile([128, 512], F32, tag="psA", bufs=2)
            nc.tensor.matmul(psn[:], tONE[:], tNSQrow[:, cc * 512:(cc + 1) * 512],
                             start=True, stop=True)
            nc.scalar.copy(tNSQ[:, cc * 512:(cc + 1) * 512], psn[:])

        # persistent per-block merge outputs for phase B
        posall = pool.tile([128, NBLK * 16], U16)
        gidxall = pool.tile([128, NBLK * 64], U16)

        # ---------------- phase A: compute + merge ----------------
        for blk in range(NBLK):
            q0 = blk * 128
            rowbuf = rowp.tile([128, N], F32, tag="rowbuf", bufs=2)

            for sub in range(4):
                c0 = sub * 2048
                psA = psum.tile([128, 2048], F32, tag="psA", bufs=2)
                for cc in range(4):
                    nc.tensor.matmul(
                        psA[:, cc * 512:(cc + 1) * 512],
                        tLQ[:, q0:q0 + 128],
                        tRC[:, c0 + cc * 512:c0 + (cc + 1) * 512],
                        start=True, stop=True,
                    )
                t1n = small.tile([128, 2048], F32, tag="t1n", bufs=2)
                nc.scalar.activation(t1n[:], psA[:],
                                     mybir.ActivationFunctionType.Identity,
                                     bias=tSQI[:, blk:blk + 1], scale=2.0)
                nc.gpsimd.tensor_tensor(rowbuf[:, c0:c0 + 2048],
                                        t1n[:], tNSQ[:, c0:c0 + 2048],
                                        op=mybir.AluOpType.subtract)

            # DVE: per-chunk top-8 values + local indices
            valbuf = small.tile([128, 64], F32, tag="valbuf")
            idxbuf = small.tile([128, 64], U16, tag="idxbuf")
            for c in range(NCH):
                nc.vector.max(valbuf[:, c * 8:(c + 1) * 8],
                              rowbuf[:, c * CH:(c + 1) * CH])
                nc.vector.max_index(idxbuf[:, c * 8:(c + 1) * 8],
                                    valbuf[:, c * 8:(c + 1) * 8],
                                    rowbuf[:, c * CH:(c + 1) * CH])

            # global candidate index = local + chunk base
            nc.vector.tensor_tensor(gidxall[:, blk * 64:(blk + 1) * 64],
                                    idxbuf[:], tCB[:], op=mybir.AluOpType.add)

            # merge: top-16 of the 64-entry buffer (tie-exact)
            mm1 = small.tile([128, 8], F32, tag="mm1")
            nc.vector.max(mm1[:], valbuf[:])
            nc.vector.max_index(posall[:, blk * 16:blk * 16 + 8], mm1[:], valbuf[:])
            vb2 = small.tile([128, 64], F32, tag="vb2")
            nc.vector.match_replace(vb2[:], mm1[:], valbuf[:], NEGBIG)
            mm2 = small.tile([128, 8], F32, tag="mm2")
            nc.vector.max(mm2[:], vb2[:])
            nc.vector.max_index(posall[:, blk * 16 + 8:blk * 16 + 16], mm2[:], vb2[:])

        # ---------------- phase B: index routing + output ----------------
        # rank_at[p] = 1+rank of buffer slot p (0 elsewhere); sidx = rank_at-1
        # is -1 for non-winners, which local_scatter SKIPS, so out16 is just
        # the 16 winner indices in rank order.
        for blk in range(NBLK):
            q0 = blk * 128
            rank_at = small.tile([128, 64], I16, tag="rank_at")
            nc.gpsimd.local_scatter(
                rank_at[:], tRK[:],
                posall[:, blk * 16:(blk + 1) * 16].bitcast(I16),
                channels=128, num_elems=64, num_idxs=16)
            s_t = small.tile([128, 64], I16, tag="s_t")
            nc.vector.tensor_scalar(s_t[:], rank_at[:], 1, None,
                                    op0=mybir.AluOpType.subtract)
            out16 = small.tile([128, K], U16, tag="out16")
            nc.gpsimd.local_scatter(
                out16[:].bitcast(I16),
                gidxall[:, blk * 64:(blk + 1) * 64].bitcast(I16), s_t[:],
                channels=128, num_elems=K, num_idxs=64)
            nc.sync.dma_start(dOUT[q0:q0 + 128, :], out16[:])

    nc.compile()
    _cache["nc"] = nc
    return nc


def _consts():
    cb = np.broadcast_to(np.repeat(np.arange(NCH, dtype=np.uint16) * CH, 8),
                         (128, 64)).copy()
    rk = np.broadcast_to(np.arange(1, 17, dtype=np.int16), (128, 16)).copy()
    return cb, rk


def _build_sides(P):
    """P [N,3] fp32 -> (XYZ [3,N], sq [N]) with sq = fl(fl(x^2+y^2)+z^2)."""
    x = P[:, 0].astype(np.float32)
    y = P[:, 1].astype(np.float32)
    z = P[:, 2].astype(np.float32)
    sq = ((x * x + y * y) + z * z).astype(np.float32)
    xyz = np.stack([x, y, z])
    return xyz, sq


def kernel(points: np.ndarray) -> np.ndarray:
    from concourse import bass_utils
    import os

    points = np.asarray(points, dtype=np.float32)
    assert points.shape == (B, N, 3), points.shape

    nc = _get_nc()
    cb, rk = _consts()

    in_maps = []
    sides = [_build_sides(points[b]) for b in range(B)]
    for core in range(8):
        b, half = core // 2, core % 2
        xyz, sq = sides[b]
        sqi = sq[half * NQ:(half + 1) * NQ].reshape(NBLK, 128).T
        in_maps.append({
            "LQ": np.ascontiguousarray(xyz[:, half * NQ:(half + 1) * NQ]),
            "RC": np.ascontiguousarray(xyz),
            "SQI": np.ascontiguousarray(-sqi),
            "NSQ": np.ascontiguousarray(sq.reshape(1, N)),
            "CB": cb, "RK": rk,
        })

    trace = os.environ.get("KNN_TRACE", "0") == "1"
    try:
        res = bass_utils.run_bass_kernel_spmd(
            nc, in_maps, core_ids=list(range(8)), trace=trace,
            trace_cores=list(range(8)) if trace else None,
        )
    except ModuleNotFoundError:
        res = bass_utils.run_bass_kernel_spmd(nc, in_maps, core_ids=list(range(8)))
    if trace:
        _cache["last_results"] = res

    out = np.empty((B, N, K), np.int32)
    for core in range(8):
        b, half = core // 2, core % 2
        out[b, half * NQ:(half + 1) * NQ, :] = res.results[core]["OUT"].astype(np.int32)
    return out


# revision 25
# speedup vs baseline: 1.1386x; 1.1386x over previous
"""KNN top-16 kernel for Trainium2 (8 NeuronCores, SPMD data-parallel).

Problem: points [4, 8192, 3] fp32 -> nn_idx [4, 8192, 16] int32
(indices of the 16 nearest neighbors by squared L2 distance, jax.lax.top_k
tie semantics: equal values ranked by ascending index).

Numerics — BIT-EXACT vs the reference backend:
The reference adj = sq + (-2 einsum) + sq^T is computed by the neuron
backend as a PE fp32 matmul (ein), then fl(-2 ein), then two left-to-right
fp32 adds.  This kernel reproduces every rounding step exactly:
  PE   : ein panel   = fp32 matmul [x,y,z]_q^T @ [x,y,z]_c   (bitwise == ref)
  ACT  : -t1         = fl(2*ein - sq_i)    Identity, scale=+2, bias=-sq_i,
                       PSUM -> SBUF        (= -fl(sq_i + inner), bitwise)
  POOL : v           = fl(-t1 - sq_j)      gpsimd tensor_tensor subtract,
                       SBUF -> SBUF        (= -fl(t1 + sq_j) = -adj, bitwise)
  DVE  : per-1024-chunk InstMax top-8 + InstMaxIndex, 64-wide merge to
         top-16 values/positions (exact jax tie semantics)
  POOL : (phase B, after one library switch) two local_scatters per block
         route global candidate indices to their rank slots
Sharding: core k handles batch k//2, query rows (k%2)*4096 ... +4096.
No collectives; full inputs in, full output gathered on host.
"""

import numpy as np
from contextlib import ExitStack

B = 4
N = 8192
K = 16
NQ = 4096          # query rows per core
CH = 1024          # candidate chunk width for DVE top-8
NCH = N // CH      # 8 chunks
NBLK = NQ // 128   # 32 blocks of 128 query rows
NEGBIG = -3.0e38

_cache = {}


def _get_nc():
    if "nc" in _cache:
        return _cache["nc"]

    import concourse.bass as bass
    import concourse.bacc as bacc
    import concourse.mybir as mybir
    import concourse.tile as tile

    F32 = mybir.dt.float32
    U16 = mybir.dt.uint16
    I16 = mybir.dt.int16
    I32 = mybir.dt.int32

    nc = bacc.Bacc("TRN2", num_devices=8)

    dLQ = nc.dram_tensor("LQ", [3, NQ], F32, kind="ExternalInput")    # x,y,z queries
    dRC = nc.dram_tensor("RC", [3, N], F32, kind="ExternalInput")     # x,y,z candidates
    dSQI = nc.dram_tensor("SQI", [128, NBLK], F32, kind="ExternalInput")  # -sq_i
    dNSQ = nc.dram_tensor("NSQ", [1, N], F32, kind="ExternalInput")   # +sq_j row
    dCB = nc.dram_tensor("CB", [128, 64], U16, kind="ExternalInput")
    dRK = nc.dram_tensor("RK", [128, 16], I16, kind="ExternalInput")
    dOUT = nc.dram_tensor("OUT", [NQ, K], U16, kind="ExternalOutput")

    with tile.TileContext(nc) as tc, ExitStack() as ctx:
        pool = ctx.enter_context(tc.tile_pool(name="pool", bufs=1))
        rowp = ctx.enter_context(tc.tile_pool(name="rowp", bufs=2))
        psum = ctx.enter_context(tc.tile_pool(name="psum", bufs=2, space="PSUM"))
        small = ctx.enter_context(tc.tile_pool(name="small", bufs=3))

        tLQ = pool.tile([3, NQ], F32)
        nc.sync.dma_start(tLQ[:], dLQ[:])
        tRC = pool.tile([3, N], F32)
        nc.sync.dma_start(tRC[:], dRC[:])
        tSQI = pool.tile([128, NBLK], F32)
        nc.sync.dma_start(tSQI[:], dSQI[:])
        # replicate +sq_j to all 128 partitions by log-doubling SBUF DMAs
        tNSQ = pool.tile([128, N], F32)
        nc.sync.dma_start(tNSQ[0:1, :], dNSQ[:])
        for k in range(7):
            p = 1 << k
            nc.sync.dma_start(tNSQ[p:2 * p, :], tNSQ[0:p, :])
        tCB = pool.tile([128, 64], U16)
        nc.sync.dma_start(tCB[:], dCB[:])
        tRK = pool.tile([128, 16], I16)
        nc.sync.dma_start(tRK[:], dRK[:])

        # persistent per-block merge outputs for phase B
        posall = pool.tile([128, NBLK * 16], U16)
        gidxall = pool.tile([128, NBLK * 64], U16)

        # ---------------- phase A: compute + merge ----------------
        for blk in range(NBLK):
            q0 = blk * 128
            rowbuf = rowp.tile([128, N], F32, tag="rowbuf", bufs=2)

            for sub in range(4):
                c0 = sub * 2048
                psA = psum.tile([128, 2048], F32, tag="psA", bufs=2)
                for cc in range(4):
                    nc.tensor.matmul(
                        psA[:, cc * 512:(cc + 1) * 512],
                        tLQ[:, q0:q0 + 128],
                        tRC[:, c0 + cc * 512:c0 + (cc + 1) * 512],
                        start=True, stop=True,
                    )
                t1n = small.tile([128, 2048], F32, tag="t1n", bufs=2)
                nc.scalar.activation(t1n[:], psA[:],
                                     mybir.ActivationFunctionType.Identity,
                                     bias=tSQI[:, blk:blk + 1], scale=2.0)
                nc.gpsimd.tensor_tensor(
                    rowbuf[:, c0:c0 + 2048], t1n[:],
                    tNSQ[:, c0:c0 + 2048],
                    op=mybir.AluOpType.subtract)

            # DVE: per-chunk top-8 values + local indices
            valbuf = small.tile([128, 64], F32, tag="valbuf")
            idxbuf = small.tile([128, 64], U16, tag="idxbuf")
            for c in range(NCH):
                nc.vector.max(valbuf[:, c * 8:(c + 1) * 8],
                              rowbuf[:, c * CH:(c + 1) * CH])
                nc.vector.max_index(idxbuf[:, c * 8:(c + 1) * 8],
                                    valbuf[:, c * 8:(c + 1) * 8],
                                    rowbuf[:, c * CH:(c + 1) * CH])

            # global candidate index = local + chunk base
            nc.vector.tensor_tensor(gidxall[:, blk * 64:(blk + 1) * 64],
                                    idxbuf[:], tCB[:], op=mybir.AluOpType.add)

            # merge: top-16 of the 64-entry buffer (tie-exact)
            mm1 = small.tile([128, 8], F32, tag="mm1")
            nc.vector.max(mm1[:], valbuf[:])
            nc.vector.max_index(posall[:, blk * 16:blk * 16 + 8], mm1[:], valbuf[:])
            vb2 = small.tile([128, 64], F32, tag="vb2")
            nc.vector.match_replace(vb2[:], mm1[:], valbuf[:], NEGBIG)
            mm2 = small.tile([128, 8], F32, tag="mm2")
            nc.vector.max(mm2[:], vb2[:])
            nc.vector.max_index(posall[:, blk * 16 + 8:blk * 16 + 16], mm2[:], vb2[:])

        # ---------------- phase B: index routing + output ----------------
        # rank_at[p] = 1+rank of buffer slot p (0 elsewhere); sidx = rank_at-1
        # is -1 for non-winners, which local_scatter SKIPS, so out16 is just
        # the 16 winner indices in rank order.  Batched: all scatter1s, one
        # subtract, all scatter2s — avoids per-block Pool<->DVE ping-pong.
        rankall = pool.tile([128, NBLK * 64], I16)
        for blk in range(NBLK):
            nc.gpsimd.local_scatter(
                rankall[:, blk * 64:(blk + 1) * 64], tRK[:],
                posall[:, blk * 16:(blk + 1) * 16].bitcast(I16),
                channels=128, num_elems=64, num_idxs=16)
        sall = pool.tile([128, NBLK * 64], I16)
        nc.vector.tensor_scalar(sall[:], rankall[:], 1, None,
                                op0=mybir.AluOpType.subtract)
        for blk in range(NBLK):
            q0 = blk * 128
            out16 = small.tile([128, K], U16, tag="out16")
            nc.gpsimd.local_scatter(
                out16[:].bitcast(I16),
                gidxall[:, blk * 64:(blk + 1) * 64].bitcast(I16),
                sall[:, blk * 64:(blk + 1) * 64],
                channels=128, num_elems=K, num_idxs=64)
            nc.sync.dma_start(dOUT[q0:q0 + 128, :], out16[:])

    nc.compile()
    _cache["nc"] = nc
    return nc


def _consts():
    cb = np.broadcast_to(np.repeat(np.arange(NCH, dtype=np.uint16) * CH, 8),
                         (128, 64)).copy()
    rk = np.broadcast_to(np.arange(1, 17, dtype=np.int16), (128, 16)).copy()
    return cb, rk


def _build_sides(P):
    """P [N,3] fp32 -> (XYZ [3,N], sq [N]) with sq = fl(fl(x^2+y^2)+z^2)."""
    x = P[:, 0].astype(np.float32)
    y = P[:, 1].astype(np.float32)
    z = P[:, 2].astype(np.float32)
    sq = ((x * x + y * y) + z * z).astype(np.float32)
    xyz = np.stack([x, y, z])
    return xyz, sq


def _get_runner(nc):
    """Cached shard_map-jitted executor (mirrors bass2jax.run_bass_via_pjrt's
    multi-core branch, but builds the jitted callable once per process —
    run_bass_via_pjrt re-traces on every call)."""
    if "runner" in _cache:
        return _cache["runner"]
    import jax
    import numpy as _np
    import concourse.mybir as mybir
    from concourse import bass2jax
    from jax.experimental.shard_map import shard_map
    from jax.sharding import Mesh, PartitionSpec

    bass2jax.install_neuronx_cc_hook()
    n_cores = 8
    partition_name = (nc.partition_id_tensor.name if nc.partition_id_tensor
                      else None)
    in_names, out_names, out_avals, zero_outs = [], [], [], []
    for alloc in nc.m.functions[0].allocations:
        if not isinstance(alloc, mybir.MemoryLocationSet):
            continue
        name = alloc.memorylocations[0].name
        if alloc.kind == "ExternalInput":
            if name != partition_name:
                in_names.append(name)
        elif alloc.kind == "ExternalOutput":
            shape = tuple(alloc.tensor_shape)
            dtype = mybir.dt.np(alloc.dtype)
            out_names.append(name)
            out_avals.append(jax.core.ShapedArray(shape, dtype))
            zero_outs.append(_np.zeros(shape, dtype))
    n_params = len(in_names)
    n_outs = len(out_names)
    all_in_names = list(in_names) + list(out_names)
    if partition_name is not None:
        all_in_names.append(partition_name)
    donate = tuple(range(n_params, n_params + n_outs))

    def _body(*args):
        operands = list(args)
        if partition_name is not None:
            operands.append(bass2jax.partition_id_tensor())
        outs = bass2jax._bass_exec_p.bind(
            *operands,
            out_avals=tuple(out_avals),
            in_names=tuple(all_in_names),
            out_names=tuple(out_names),
            lowering_input_output_aliases=(),
            sim_require_finite=True,
            sim_require_nnan=True,
            nc=nc,
        )
        return tuple(outs)

    devices = jax.devices()[:n_cores]
    assert len(devices) == n_cores, f"need {n_cores} devices"
    mesh = Mesh(_np.asarray(devices), ("core",))
    in_specs = (PartitionSpec("core"),) * (n_params + n_outs)
    out_specs = (PartitionSpec("core"),) * n_outs
    sharded = jax.jit(
        shard_map(_body, mesh=mesh, in_specs=in_specs, out_specs=out_specs,
                  check_rep=False),
        donate_argnums=donate, keep_unused=True,
    )

    def runner(in_maps):
        per_core = [[_np.asarray(m[name]) for name in in_names] for m in in_maps]
        concat_in = [
            _np.concatenate([per_core[c][i] for c in range(n_cores)], axis=0)
            for i in range(n_params)
        ]
        concat_zeros = [
            _np.zeros((n_cores * z.shape[0], *z.shape[1:]), z.dtype)
            for z in zero_outs
        ]
        out_arrs = sharded(*concat_in, *concat_zeros)
        return [
            {name: _np.asarray(out_arrs[i]).reshape(n_cores, *out_avals[i].shape)[c]
             for i, name in enumerate(out_names)}
            for c in range(n_cores)
        ]

    _cache["runner"] = runner
    return runner


def kernel(points: np.ndarray) -> np.ndarray:
    from concourse import bass_utils
    import os

    points = np.asarray(points, dtype=np.float32)
    assert points.shape == (B, N, 3), points.shape

    nc = _get_nc()
    cb, rk = _consts()

    in_maps = []
    sides = [_build_sides(points[b]) for b in range(B)]
    for core in range(8):
        b, half = core // 2, core % 2
        xyz, sq = sides[b]
        sqi = sq[half * NQ:(half + 1) * NQ].reshape(NBLK, 128).T
        in_maps.append({
            "LQ": np.ascontiguousarray(xyz[:, half * NQ:(half + 1) * NQ]),
            "RC": np.ascontiguousarray(xyz),
            "SQI": np.ascontiguousarray(-sqi),
            "NSQ": np.ascontiguousarray(sq.reshape(1, N)),
            "CB": cb, "RK": rk,
        })

    trace = os.environ.get("KNN_TRACE", "0") == "1"
    try:
        res = bass_utils.run_bass_kernel_spmd(
            nc, in_maps, core_ids=list(range(8)), trace=trace,
            trace_cores=list(range(8)) if trace else None,
        )
    except ModuleNotFoundError:
        res = bass_utils.run_bass_kernel_spmd(nc, in_maps, core_ids=list(range(8)))
    if trace:
        _cache["last_results"] = res

    out = np.empty((B, N, K), np.int32)
    for core in range(8):
        b, half = core // 2, core % 2
        out[b, half * NQ:(half + 1) * NQ, :] = res.results[core]["OUT"].astype(np.int32)
    return out


# revision 26
# speedup vs baseline: 3.5643x; 3.1306x over previous
"""KNN top-16 kernel for Trainium2 (8 NeuronCores, SPMD data-parallel).

Problem: points [4, 8192, 3] fp32 -> nn_idx [4, 8192, 16] int32
(indices of the 16 nearest neighbors by squared L2 distance, jax.lax.top_k
tie semantics: equal values ranked by ascending index).

Numerics — BIT-EXACT vs the reference backend:
The reference adj = sq + (-2 einsum) + sq^T is computed by the neuron
backend as a PE fp32 matmul (ein), then fl(-2 ein), then two left-to-right
fp32 adds.  This kernel reproduces every rounding step exactly:
  PE   : ein panel   = fp32 matmul [x,y,z]_q^T @ [x,y,z]_c   (bitwise == ref)
  ACT  : -t1         = fl(2*ein - sq_i)    Identity, scale=+2, bias=-sq_i,
                       PSUM -> SBUF        (= -fl(sq_i + inner), bitwise)
  POOL : v           = fl(-t1 - sq_j)      gpsimd tensor_tensor subtract,
                       SBUF -> SBUF        (= -fl(t1 + sq_j) = -adj, bitwise)
  DVE  : per-1024-chunk InstMax top-8 + InstMaxIndex, 64-wide merge to
         top-16 values/positions (exact jax tie semantics)
  POOL : (phase B, after one library switch) two local_scatters per block
         route global candidate indices to their rank slots
Sharding: core k handles batch k//2, query rows (k%2)*4096 ... +4096.
No collectives; full inputs in, full output gathered on host.
"""

import numpy as np
from contextlib import ExitStack

B = 4
N = 8192
K = 16
NQ = 4096          # query rows per core
CH = 1024          # candidate chunk width for DVE top-8
NCH = N // CH      # 8 chunks
NBLK = NQ // 128   # 32 blocks of 128 query rows
NEGBIG = -3.0e38

_cache = {}


def _get_nc():
    if "nc" in _cache:
        return _cache["nc"]

    import concourse.bass as bass
    import concourse.bacc as bacc
    import concourse.mybir as mybir
    import concourse.tile as tile

    F32 = mybir.dt.float32
    U16 = mybir.dt.uint16
    I16 = mybir.dt.int16
    I32 = mybir.dt.int32

    nc = bacc.Bacc("TRN2", num_devices=8)

    dLQ = nc.dram_tensor("LQ", [3, NQ], F32, kind="ExternalInput")    # x,y,z queries
    dRC = nc.dram_tensor("RC", [3, N], F32, kind="ExternalInput")     # x,y,z candidates
    dSQI = nc.dram_tensor("SQI", [128, NBLK], F32, kind="ExternalInput")  # -sq_i
    dNSQ = nc.dram_tensor("NSQ", [1, N], F32, kind="ExternalInput")   # +sq_j row
    dCB = nc.dram_tensor("CB", [128, 64], U16, kind="ExternalInput")
    dRK = nc.dram_tensor("RK", [128, 16], I16, kind="ExternalInput")
    dOUT = nc.dram_tensor("OUT", [NQ, K], U16, kind="ExternalOutput")

    with tile.TileContext(nc) as tc, ExitStack() as ctx:
        pool = ctx.enter_context(tc.tile_pool(name="pool", bufs=1))
        rowp = ctx.enter_context(tc.tile_pool(name="rowp", bufs=2))
        psum = ctx.enter_context(tc.tile_pool(name="psum", bufs=2, space="PSUM"))
        small = ctx.enter_context(tc.tile_pool(name="small", bufs=3))

        tLQ = pool.tile([3, NQ], F32)
        nc.sync.dma_start(tLQ[:], dLQ[:])
        tRC = pool.tile([3, N], F32)
        nc.sync.dma_start(tRC[:], dRC[:])
        tSQI = pool.tile([128, NBLK], F32)
        nc.sync.dma_start(tSQI[:], dSQI[:])
        # replicate +sq_j to all 128 partitions by log-doubling SBUF DMAs
        tNSQ = pool.tile([128, N], F32)
        nc.sync.dma_start(tNSQ[0:1, :], dNSQ[:])
        for k in range(7):
            p = 1 << k
            nc.sync.dma_start(tNSQ[p:2 * p, :], tNSQ[0:p, :])
        tCB = pool.tile([128, 64], U16)
        nc.sync.dma_start(tCB[:], dCB[:])
        tRK = pool.tile([128, 16], I16)
        nc.sync.dma_start(tRK[:], dRK[:])

        # persistent per-block merge outputs for phase B
        posall = pool.tile([128, NBLK * 16], U16)
        gidxall = pool.tile([128, NBLK * 64], U16)

        # ---------------- phase A: compute + merge ----------------
        for blk in range(NBLK):
            q0 = blk * 128
            rowbuf = rowp.tile([128, N], F32, tag="rowbuf", bufs=2)

            for sub in range(4):
                c0 = sub * 2048
                psA = psum.tile([128, 2048], F32, tag="psA", bufs=2)
                for cc in range(4):
                    nc.tensor.matmul(
                        psA[:, cc * 512:(cc + 1) * 512],
                        tLQ[:, q0:q0 + 128],
                        tRC[:, c0 + cc * 512:c0 + (cc + 1) * 512],
                        start=True, stop=True,
                    )
                t1n = small.tile([128, 2048], F32, tag="t1n", bufs=2)
                nc.scalar.activation(t1n[:], psA[:],
                                     mybir.ActivationFunctionType.Identity,
                                     bias=tSQI[:, blk:blk + 1], scale=2.0)
                nc.gpsimd.tensor_tensor(
                    rowbuf[:, c0:c0 + 2048], t1n[:],
                    tNSQ[:, c0:c0 + 2048],
                    op=mybir.AluOpType.subtract)

            # DVE: per-chunk top-8 values + local indices
            valbuf = small.tile([128, 64], F32, tag="valbuf")
            idxbuf = small.tile([128, 64], U16, tag="idxbuf")
            for c in range(NCH):
                nc.vector.max(valbuf[:, c * 8:(c + 1) * 8],
                              rowbuf[:, c * CH:(c + 1) * CH])
                nc.vector.max_index(idxbuf[:, c * 8:(c + 1) * 8],
                                    valbuf[:, c * 8:(c + 1) * 8],
                                    rowbuf[:, c * CH:(c + 1) * CH])

            # global candidate index = local + chunk base
            nc.vector.tensor_tensor(gidxall[:, blk * 64:(blk + 1) * 64],
                                    idxbuf[:], tCB[:], op=mybir.AluOpType.add)

            # merge: top-16 of the 64-entry buffer (tie-exact)
            mm1 = small.tile([128, 8], F32, tag="mm1")
            nc.vector.max(mm1[:], valbuf[:])
            nc.vector.max_index(posall[:, blk * 16:blk * 16 + 8], mm1[:], valbuf[:])
            vb2 = small.tile([128, 64], F32, tag="vb2")
            nc.vector.match_replace(vb2[:], mm1[:], valbuf[:], NEGBIG)
            mm2 = small.tile([128, 8], F32, tag="mm2")
            nc.vector.max(mm2[:], vb2[:])
            nc.vector.max_index(posall[:, blk * 16 + 8:blk * 16 + 16], mm2[:], vb2[:])

        # ---------------- phase B: index routing + output ----------------
        # rank_at[p] = 1+rank of buffer slot p (0 elsewhere); sidx = rank_at-1
        # is -1 for non-winners, which local_scatter SKIPS, so out16 is just
        # the 16 winner indices in rank order.  Batched: all scatter1s, one
        # subtract, all scatter2s — avoids per-block Pool<->DVE ping-pong.
        rankall = pool.tile([128, NBLK * 64], I16)
        for blk in range(NBLK):
            nc.gpsimd.local_scatter(
                rankall[:, blk * 64:(blk + 1) * 64], tRK[:],
                posall[:, blk * 16:(blk + 1) * 16].bitcast(I16),
                channels=128, num_elems=64, num_idxs=16)
        sall = pool.tile([128, NBLK * 64], I16)
        nc.vector.tensor_scalar(sall[:], rankall[:], 1, None,
                                op0=mybir.AluOpType.subtract)
        for blk in range(NBLK):
            q0 = blk * 128
            out16 = small.tile([128, K], U16, tag="out16")
            nc.gpsimd.local_scatter(
                out16[:].bitcast(I16),
                gidxall[:, blk * 64:(blk + 1) * 64].bitcast(I16),
                sall[:, blk * 64:(blk + 1) * 64],
                channels=128, num_elems=K, num_idxs=64)
            nc.sync.dma_start(dOUT[q0:q0 + 128, :], out16[:])

    nc.compile()
    _cache["nc"] = nc
    return nc


def _consts():
    cb = np.broadcast_to(np.repeat(np.arange(NCH, dtype=np.uint16) * CH, 8),
                         (128, 64)).copy()
    rk = np.broadcast_to(np.arange(1, 17, dtype=np.int16), (128, 16)).copy()
    return cb, rk


def _build_sides(P):
    """P [N,3] fp32 -> (XYZ [3,N], sq [N]) with sq = fl(fl(x^2+y^2)+z^2)."""
    x = P[:, 0].astype(np.float32)
    y = P[:, 1].astype(np.float32)
    z = P[:, 2].astype(np.float32)
    sq = ((x * x + y * y) + z * z).astype(np.float32)
    xyz = np.stack([x, y, z])
    return xyz, sq


def _get_runner(nc):
    """Cached shard_map-jitted executor (mirrors bass2jax.run_bass_via_pjrt's
    multi-core branch, but builds the jitted callable once per process —
    run_bass_via_pjrt re-traces on every call)."""
    if "runner" in _cache:
        return _cache["runner"]
    import jax
    import numpy as _np
    import concourse.mybir as mybir
    from concourse import bass2jax
    from jax.experimental.shard_map import shard_map
    from jax.sharding import Mesh, PartitionSpec

    bass2jax.install_neuronx_cc_hook()
    n_cores = 8
    partition_name = (nc.partition_id_tensor.name if nc.partition_id_tensor
                      else None)
    in_names, out_names, out_avals, zero_outs = [], [], [], []
    for alloc in nc.m.functions[0].allocations:
        if not isinstance(alloc, mybir.MemoryLocationSet):
            continue
        name = alloc.memorylocations[0].name
        if alloc.kind == "ExternalInput":
            if name != partition_name:
                in_names.append(name)
        elif alloc.kind == "ExternalOutput":
            shape = tuple(alloc.tensor_shape)
            dtype = mybir.dt.np(alloc.dtype)
            out_names.append(name)
            out_avals.append(jax.core.ShapedArray(shape, dtype))
            zero_outs.append(_np.zeros(shape, dtype))
    n_params = len(in_names)
    n_outs = len(out_names)
    all_in_names = list(in_names) + list(out_names)
    if partition_name is not None:
        all_in_names.append(partition_name)
    donate = tuple(range(n_params, n_params + n_outs))

    def _body(*args):
        operands = list(args)
        if partition_name is not None:
            operands.append(bass2jax.partition_id_tensor())
        outs = bass2jax._bass_exec_p.bind(
            *operands,
            out_avals=tuple(out_avals),
            in_names=tuple(all_in_names),
            out_names=tuple(out_names),
            lowering_input_output_aliases=(),
            sim_require_finite=True,
            sim_require_nnan=True,
            nc=nc,
        )
        return tuple(outs)

    devices = jax.devices()[:n_cores]
    assert len(devices) == n_cores, f"need {n_cores} devices"
    mesh = Mesh(_np.asarray(devices), ("core",))
    in_specs = (PartitionSpec("core"),) * (n_params + n_outs)
    out_specs = (PartitionSpec("core"),) * n_outs
    sharded = jax.jit(
        shard_map(_body, mesh=mesh, in_specs=in_specs, out_specs=out_specs,
                  check_rep=False),
        donate_argnums=donate, keep_unused=True,
    )

    def runner(in_maps):
        per_core = [[_np.asarray(m[name]) for name in in_names] for m in in_maps]
        concat_in = [
            _np.concatenate([per_core[c][i] for c in range(n_cores)], axis=0)
            for i in range(n_params)
        ]
        concat_zeros = [
            _np.zeros((n_cores * z.shape[0], *z.shape[1:]), z.dtype)
            for z in zero_outs
        ]
        out_arrs = sharded(*concat_in, *concat_zeros)
        return [
            {name: _np.asarray(out_arrs[i]).reshape(n_cores, *out_avals[i].shape)[c]
             for i, name in enumerate(out_names)}
            for c in range(n_cores)
        ]

    _cache["runner"] = runner
    return runner


def kernel(points: np.ndarray) -> np.ndarray:
    from concourse import bass_utils
    import os

    points = np.asarray(points, dtype=np.float32)
    assert points.shape == (B, N, 3), points.shape

    nc = _get_nc()
    cb, rk = _consts()

    in_maps = []
    sides = [_build_sides(points[b]) for b in range(B)]
    for core in range(8):
        b, half = core // 2, core % 2
        xyz, sq = sides[b]
        sqi = sq[half * NQ:(half + 1) * NQ].reshape(NBLK, 128).T
        in_maps.append({
            "LQ": np.ascontiguousarray(xyz[:, half * NQ:(half + 1) * NQ]),
            "RC": np.ascontiguousarray(xyz),
            "SQI": np.ascontiguousarray(-sqi),
            "NSQ": np.ascontiguousarray(sq.reshape(1, N)),
            "CB": cb, "RK": rk,
        })

    trace = os.environ.get("KNN_TRACE", "0") == "1"
    if trace:
        try:
            res = bass_utils.run_bass_kernel_spmd(
                nc, in_maps, core_ids=list(range(8)), trace=True,
                trace_cores=list(range(8)),
            )
        except ModuleNotFoundError:
            res = bass_utils.run_bass_kernel_spmd(nc, in_maps,
                                                  core_ids=list(range(8)))
        _cache["last_results"] = res
        results = res.results
    else:
        results = _get_runner(nc)(in_maps)

    out = np.empty((B, N, K), np.int32)
    for core in range(8):
        b, half = core // 2, core % 2
        out[b, half * NQ:(half + 1) * NQ, :] = results[core]["OUT"].astype(np.int32)
    return out
